# revision 30
# baseline (speedup 1.0000x reference)
"""Causal self-attention (RMS-normed QK, RoPE, GQA) Trainium2 Bass kernel.

Sharding over 8 NeuronCores: 4-way data-parallel over batch x 2-way
tensor-parallel over heads.  Core c handles batch b = c // 2 and head group
g = c % 2 (q heads g*8..g*8+7, kv heads g*2, g*2+1).  Each core produces a
partial output projection; the host sums the two head-group partials per
batch.

v2 design (bf16 data path, fp32 PSUM accumulation everywhere):
  - Host pre-transposes/packs x^T and all weights into the SBUF-native
    [128, ...] partition-major layout, so every DMA is a single contiguous
    [128, N] copy (no on-device transposes of x, no rearrange DMAs).
  - RoPE via a constant +-1 half-swap matrix on the PE (one extra matmul
    per tile) instead of SBUF-SBUF DMAs.
  - RMS-norm: sum-of-squares via ones-matmul to a [1,512] row, sqrt on the
    scalar engine, reciprocal on vector, broadcast back over partitions via
    a K=1 matmul; the normalize multiply runs after RoPE (rope commutes
    with per-column scales), so one vector multiply total.
  - q_gain/sqrt(hd) ride in the per-head `scale` operand of the Exp.
  - Softmax denominators accumulate on the PE (ones-column matmuls into a
    [1,512] PSUM region) instead of vector-engine adds.
  - y^T stays in SBUF in bf16 and feeds the output projection directly as
    the stationary operand (no DRAM spill).
"""

import math

import numpy as np
import ml_dtypes

import concourse.bass as bass
import concourse.mybir as mybir
import concourse.tile as tile
from concourse import bacc, bass_utils
from concourse.bass_isa import ReduceOp
from concourse.masks import make_identity

F32 = mybir.dt.float32
F32R = mybir.dt.float32r
BF16 = mybir.dt.bfloat16
BF16NP = ml_dtypes.bfloat16

HEAD_DIM = 128
N_HEADS = 16
N_KV_HEADS = 4
ROPE_BASE = 10000.0
TRAIN_SEQ_LEN = 1024

B, D = 4, 2048
H_LOC = 8  # q heads per core
KV_LOC = 2  # kv heads per core
EC = D // 128  # contraction chunks
EPS = float(np.finfo(np.float32).eps)
INV_SQRT_HD = 1.0 / math.sqrt(HEAD_DIM)
AF = mybir.ActivationFunctionType


def _rope_tables(T):
    rd = HEAD_DIM
    base = ROPE_BASE
    if T > TRAIN_SEQ_LEN:
        scale = T / TRAIN_SEQ_LEN
        base = base * scale ** (rd / (rd - 2))
    inv_freq = 1.0 / base ** (np.arange(0, rd, 2, dtype=np.float32) / rd)
    freqs = np.outer(np.arange(T, dtype=np.float32), inv_freq)
    return np.cos(freqs).astype(np.float32), np.sin(freqs).astype(np.float32)


def _blob_layout(T):
    """(name, n_bf16_elements) regions of the packed input blob."""
    return [
        ("xt", 128 * EC * T),
        ("qwt", 128 * H_LOC * EC * 128),
        ("kwt", 128 * KV_LOC * EC * 128),
        ("vwt", 128 * KV_LOC * EC * 128),
        ("owt", 128 * H_LOC * D),
        ("cos2", 128 * T),
        ("sin2", 128 * T),  # sign-folded: lower half +sin, upper half -sin
        ("gains", 2 * H_LOC),  # H_LOC f32 values as raw bf16 pairs
    ]


def build_program(T=2048, phases=(1, 2, 3)):
    """Build the per-core Bass program. T must be a multiple of 512."""
    assert T % 512 == 0
    NT = T // 128  # 128-wide t tiles
    NTB = T // 512  # projection column chunks
    NIB = T // 512  # attention i blocks

    nc = bacc.Bacc("TRN2", target_bir_lowering=False, debug=False, num_devices=8)

    # All inputs live in ONE flat bf16 blob (a single runtime buffer per
    # call is measurably cheaper to dispatch through the runtime than ten).
    sizes = _blob_layout(T)
    total = sum(n for _, n in sizes)
    blob_d = nc.dram_tensor("blob", [total], BF16, kind="ExternalInput").ap()
    regions = {}
    off = 0
    for name, n in sizes:
        regions[name] = blob_d[off:off + n]
        off += n

    def blob_ap(name, free_shape):
        """region as a [128, *free_shape] partition-major AP (C order)."""
        r = regions[name]
        dims = list(free_shape)
        strides = []
        s = 1
        for d in reversed(dims):
            strides.append((s, d))
            s *= d
        strides.reverse()
        ap = [[s, 128]] + [[st, d] for st, d in strides]
        return bass.AP(tensor=r.tensor, offset=r.offset, ap=ap)

    xt_d = blob_ap("xt", (EC, T))
    qwt_d = blob_ap("qwt", (H_LOC, EC, 128))
    kwt_d = blob_ap("kwt", (KV_LOC, EC, 128))
    vwt_d = blob_ap("vwt", (KV_LOC, EC, 128))
    owt_d = blob_ap("owt", (H_LOC, D))
    cos_d = blob_ap("cos2", (T,))
    sin_d = blob_ap("sin2", (T,))
    g_r = regions["gains"]  # [2*H_LOC] bf16 holding H_LOC f32 values
    gains_d = bass.AP(tensor=g_r.tensor, offset=g_r.offset,
                      ap=[[0, 128], [1, 2 * H_LOC]])
    out_d = nc.dram_tensor("out", [T, D], F32, kind="ExternalOutput").ap()

    with tile.TileContext(nc) as tc:
        with (
            tc.tile_pool(name="const", bufs=1) as const_p,
            tc.tile_pool(name="pers", bufs=1) as pers_p,
        ):
            ident = const_p.tile([128, 128], BF16)
            make_identity(nc, ident)
            ones_p = const_p.tile([128, 1], BF16)  # lhsT for partition-sum
            nc.vector.memset(ones_p, 1.0)
            eps_sb = const_p.tile([128, 1], F32)
            nc.vector.memset(eps_sb, EPS)
            cos_sb = const_p.tile([128, T], BF16)
            sin_sb = const_p.tile([128, T], BF16)
            graw_sb = const_p.tile([128, 2 * H_LOC], BF16)
            gscl_sb = const_p.tile([128, H_LOC], F32)
            # causal 0/1 masks for the 4 diagonal j-tiles of an i-block:
            # mask[p, v, c] = 1 iff c >= 128*v + p
            mask_sb = const_p.tile([128, 4, 512], BF16)
            nc.vector.memset(mask_sb, 1.0)
            nc.gpsimd.affine_select(
                out=mask_sb, in_=mask_sb,
                compare_op=mybir.AluOpType.is_ge, fill=0.0,
                base=0, channel_multiplier=-1,
                pattern=[[-128, 4], [1, 512]])

            qT = pers_p.tile([128, H_LOC, T], BF16)
            kT = pers_p.tile([128, KV_LOC, T], BF16)
            v_sb = pers_p.tile([128, NT, KV_LOC * 128], BF16)
            yT = pers_p.tile([128, H_LOC, T], BF16)

            # ---------------- Phase 1: projections -----------------------
            with (
                tc.tile_pool(name="p1xt", bufs=1) as xt_p,
                tc.tile_pool(name="p1w", bufs=2) as w_p,
                tc.tile_pool(name="p1wk", bufs=3) as wk_p,
                tc.tile_pool(name="p1vt", bufs=1) as vt_p,
                tc.tile_pool(name="p1row", bufs=2) as row_p,
                tc.tile_pool(name="p1psh", bufs=4, space="PSUM") as ps_h,
                tc.tile_pool(name="p1psq", bufs=3, space="PSUM") as ps_q,
            ):
                def load_w(w_dram, idx):
                    wt = w_p.tile([128, EC, 128], BF16, tag="w")
                    nc.sync.dma_start(wt, w_dram[:, idx, :, :])
                    return wt

                # first weight before the big constants, so the first
                # projection matmul is not queued behind cos/sin
                wt_first = load_w(kwt_d, 0)

                xt = xt_p.tile([128, EC, T], BF16)
                # all xt loads on the scalar HWDGE ring in e-quarters so
                # projections start as soon as their contraction slices
                # land; Pool stays free for partition_all_reduce
                for tb in range(NTB):
                    tsl = slice(tb * 512, (tb + 1) * 512)
                    for part in range(4):
                        esl = slice(4 * part, 4 * part + 4)
                        nc.scalar.dma_start(xt[:, esl, tsl], xt_d[:, esl, tsl])

                nc.sync.dma_start(cos_sb, cos_d)
                nc.sync.dma_start(sin_sb, sin_d)
                # gains arrive as raw f32 bits inside the bf16 blob;
                # broadcast to all partitions via stride-0 DMA, then scale.
                nc.sync.dma_start(graw_sb, gains_d)
                nc.scalar.mul(gscl_sb, graw_sb.bitcast(F32), INV_SQRT_HD)

                def project_chunk(wt, tsl):
                    h_ps = ps_h.tile([128, 512], F32, tag="hps")
                    for e in range(EC):
                        nc.tensor.matmul(h_ps, wt[:, e, :], xt[:, e, tsl],
                                         start=(e == 0), stop=(e == EC - 1))
                    return h_ps

                def norm_rope_chunk(h_ps, tsl, dst):
                    """dst = rms_norm+rope of the raw projection chunk.

                    No PE instructions: the partition-sum runs on GPSIMD,
                    the rope half-swap is an SBUF-SBUF DMA across
                    partitions (sign folded into the sin table), so the PE
                    stream is pure projection matmuls.
                    """
                    x_sb = wk_p.tile([128, 512], BF16, tag="xsb")
                    nc.scalar.copy(x_sb, h_ps)
                    sq = wk_p.tile([128, 512], BF16, tag="sq")
                    nc.scalar.square(sq, h_ps)
                    ssq = row_p.tile([128, 512], F32, tag="ssq")
                    nc.gpsimd.partition_all_reduce(ssq, sq, 128, ReduceOp.add)
                    rms = wk_p.tile([128, 512], BF16, tag="rms")
                    with nc.allow_low_precision(reason="bf16 norm scale"):
                        nc.scalar.activation(rms, ssq, AF.Sqrt,
                                             bias=eps_sb, scale=1.0 / 128.0)
                    rinv = wk_p.tile([128, 512], BF16, tag="rinv")
                    with nc.allow_low_precision(reason="bf16 norm scale"):
                        nc.vector.reciprocal(rinv, rms)
                    # rope: rot = x*cos + swap64(x)*sin', sin' sign-folded
                    xsw = wk_p.tile([128, 512], BF16, tag="xsw")
                    nc.sync.dma_start(xsw[0:64, :], x_sb[64:128, :])
                    nc.sync.dma_start(xsw[64:128, :], x_sb[0:64, :])
                    rc = wk_p.tile([128, 512], BF16, tag="rc")
                    nc.vector.tensor_mul(rc, x_sb, cos_sb[:, tsl])
                    qsw = wk_p.tile([128, 512], BF16, tag="qsw")
                    nc.vector.tensor_mul(qsw, xsw, sin_sb[:, tsl])
                    qr = wk_p.tile([128, 512], BF16, tag="qr")
                    nc.vector.tensor_add(qr, rc, qsw)
                    nc.vector.tensor_mul(dst, qr, rinv)

                for kv in range(KV_LOC):
                    # K and V chunks interleave per column block so the PE
                    # has ~7us of work per 6.3us xt column-block DMA during
                    # the cold start.  V transposes trail after each kv
                    # head, overlapping the next head's projections.
                    wt = wt_first if kv == 0 else load_w(kwt_d, kv)
                    wtv = load_w(vwt_d, kv)
                    vts = []
                    for tb in range(NTB):
                        tsl = slice(tb * 512, (tb + 1) * 512)
                        h_ps = project_chunk(wt, tsl)
                        norm_rope_chunk(h_ps, tsl, kT[:, kv, tsl])
                        v_ps = project_chunk(wtv, tsl)
                        vt = vt_p.tile([128, 512], BF16, tag=f"vt{tb}")
                        nc.vector.tensor_copy(vt, v_ps)
                        vts.append(vt)
                    for tb in range(NTB):
                        for tt in range(4):
                            pst = ps_q.tile([128, 128], BF16, tag="qsps")
                            nc.tensor.transpose(
                                pst, vts[tb][:, tt * 128:(tt + 1) * 128],
                                ident)
                            nc.vector.tensor_copy(
                                v_sb[:, tb * 4 + tt,
                                     kv * 128:(kv + 1) * 128], pst)

                for h in range(H_LOC):
                    wt = load_w(qwt_d, h)
                    for tb in range(NTB):
                        tsl = slice(tb * 512, (tb + 1) * 512)
                        h_ps = project_chunk(wt, tsl)
                        norm_rope_chunk(h_ps, tsl, qT[:, h, tsl])

            # out_w prefetch: pool opens after phase-1 pools free their
            # SBUF, DMA overlaps the whole attention phase.
            with tc.tile_pool(name="p3ow", bufs=1) as ow_p:
                ow_sb = ow_p.tile([128, H_LOC, D], BF16)
                nc.sync.dma_start(ow_sb, owt_d)

                # ------- Phase 2+3: attention + output projection ---------
                # ib-major attention; after each i-block completes for all
                # heads, the output projection for those 4 i-tiles is
                # issued.  The projection matmuls sit in the PE queue with
                # all inputs ready, filling the softmax-chain stalls at
                # block boundaries, and the output DMA streams throughout
                # instead of all at the end.  o_ps shares the y-pool slots
                # (same shape/tag) to stay within 8 PSUM banks.
                with (
                    tc.tile_pool(name="p2pt", bufs=4) as pt_p,
                    tc.tile_pool(name="p2y", bufs=2) as ystg_p,
                    tc.tile_pool(name="p2row", bufs=2) as row2_p,
                    tc.tile_pool(name="p3o", bufs=2) as ostg_p,
                    tc.tile_pool(name="p2pss", bufs=4, space="PSUM") as ps_s,
                    tc.tile_pool(name="p2psy", bufs=2, space="PSUM") as ps_y,
                    tc.tile_pool(name="p2psl", bufs=2, space="PSUM") as ps_l,
                ):
                    for ib in range(NIB if 2 in phases else 0):
                        jmax = 4 * ib + 3
                        isl = slice(ib * 512, (ib + 1) * 512)
                        for h in range(H_LOC):
                            kv = h // (N_HEADS // N_KV_HEADS)
                            y_ps = ps_y.tile([128, 512], F32, tag="y")
                            l_ps = ps_l.tile([1, 512], F32, tag="l")
                            for jt in range(jmax + 1):
                                # diagonal j-tiles: columns below the causal
                                # front are all-masked; skip them entirely
                                v = jt - 4 * ib
                                c0 = 128 * v if v > 0 else 0
                                cols = slice(c0, 512)
                                qsl = slice(ib * 512 + c0, (ib + 1) * 512)
                                s_ps = ps_s.tile([128, 512], F32, tag="s")
                                nc.tensor.matmul(
                                    s_ps[:, cols],
                                    kT[:, kv, jt * 128:(jt + 1) * 128],
                                    qT[:, h, qsl], start=True, stop=True)
                                pt = pt_p.tile([128, 512], BF16, tag="pt")
                                nc.scalar.activation(
                                    pt[:, cols], s_ps[:, cols], AF.Exp,
                                    scale=gscl_sb[:, h:h + 1])
                                ptu = pt
                                if v >= 0:  # diagonal j-tile: causal mask
                                    ptm = pt_p.tile([128, 512], BF16,
                                                    tag="ptm")
                                    nc.vector.tensor_mul(
                                        ptm[:, cols], pt[:, cols],
                                        mask_sb[:, v, cols])
                                    ptu = ptm
                                nc.tensor.matmul(
                                    l_ps[:, cols], ones_p, ptu[:, cols],
                                    start=(jt == 0), stop=(jt == jmax))
                                nc.tensor.matmul(
                                    y_ps[:, cols],
                                    v_sb[:, jt, kv * 128:(kv + 1) * 128],
                                    ptu[:, cols],
                                    start=(jt == 0), stop=(jt == jmax))
                            lrow = row2_p.tile([1, 512], BF16, tag="lr")
                            with nc.allow_low_precision(reason="bf16 softmax"):
                                nc.vector.reciprocal(lrow, l_ps)
                            linv_bc = ystg_p.tile([128, 512], BF16, tag="linv")
                            nc.gpsimd.partition_broadcast(linv_bc, lrow)
                            nc.vector.tensor_mul(yT[:, h, isl], y_ps, linv_bc)

                        if 3 not in phases:
                            continue
                        for it in range(4 * ib, 4 * ib + 4):
                            o_sb = ostg_p.tile([128, D], F32, tag="osb")
                            for db in range(D // 512):
                                o_ps = ps_y.tile([128, 512], F32, tag="y")
                                for hh in range(H_LOC):
                                    nc.tensor.matmul(
                                        o_ps,
                                        yT[:, hh, it * 128:(it + 1) * 128],
                                        ow_sb[:, hh,
                                              db * 512:(db + 1) * 512],
                                        start=(hh == 0),
                                        stop=(hh == H_LOC - 1))
                                nc.vector.tensor_copy(
                                    o_sb[:, db * 512:(db + 1) * 512], o_ps)
                            nc.sync.dma_start(
                                out_d[it * 128:(it + 1) * 128, :], o_sb)

    nc.compile()
    return nc


def _pack_weight(w):
    """w [ncols, D]: returns [128, nh, EC, 128] with
    out[p, h, e, c] = w[h*128 + c, e*128 + p]."""
    nh = w.shape[0] // 128
    return np.ascontiguousarray(
        w.reshape(nh, 128, EC, 128).transpose(3, 0, 2, 1)).astype(BF16NP)


def make_in_maps(x, q_w, k_w, v_w, out_w, q_gain, T):
    cos, sin = _rope_tables(T)
    cosT = np.ascontiguousarray(cos.T)  # [64, T]
    sinT = np.ascontiguousarray(sin.T)
    cos2 = np.concatenate([cosT, cosT], axis=0).astype(BF16NP)  # [128, T]
    # rope sign folded into the table: rot = x*cos2 + swap64(x)*sin2
    sin2 = np.concatenate([sinT, -sinT], axis=0).astype(BF16NP)

    layout = _blob_layout(T)
    in_maps = []
    for c in range(8):
        b, g = c // 2, c % 2
        # xt[p, e, t] = x[b][t, e*128+p]
        xtp = np.ascontiguousarray(
            x[b].T.reshape(EC, 128, T).transpose(1, 0, 2)).astype(BF16NP)
        # ow[p, h, d] = out_w[d, g*1024 + h*128 + p]
        owp = np.ascontiguousarray(
            out_w[:, g * 1024:(g + 1) * 1024].T
            .reshape(H_LOC, 128, D).transpose(1, 0, 2)).astype(BF16NP)
        parts = {
            "xt": xtp,
            "qwt": _pack_weight(q_w[g * 1024:(g + 1) * 1024, :]),
            "kwt": _pack_weight(k_w[g * 256:(g + 1) * 256, :]),
            "vwt": _pack_weight(v_w[g * 256:(g + 1) * 256, :]),
            "owt": owp,
            "cos2": cos2,
            "sin2": sin2,
            "gains": np.ascontiguousarray(
                q_gain[g * H_LOC:(g + 1) * H_LOC]).astype(np.float32)
                .view(BF16NP),
        }
        blob = np.concatenate(
            [np.asarray(parts[name]).reshape(-1) for name, _ in layout])
        for (name, n), arr in zip(layout, [parts[n] for n, _ in layout]):
            assert np.asarray(arr).size == n, (name, np.asarray(arr).size, n)
        in_maps.append({"blob": blob})
    return in_maps


def kernel(x, q_w, k_w, v_w, out_w, q_gain, _trace=False, _trace_cores=None):
    x = np.asarray(x, dtype=np.float32)
    q_w = np.asarray(q_w, dtype=np.float32)
    k_w = np.asarray(k_w, dtype=np.float32)
    v_w = np.asarray(v_w, dtype=np.float32)
    out_w = np.asarray(out_w, dtype=np.float32)
    q_gain = np.asarray(q_gain, dtype=np.float32)
    T = x.shape[1]

    nc = build_program(T)
    in_maps = make_in_maps(x, q_w, k_w, v_w, out_w, q_gain, T)
    res = bass_utils.run_bass_kernel_spmd(
        nc, in_maps, core_ids=list(range(8)),
        trace=_trace, trace_cores=_trace_cores)
    outs = [r["out"] for r in res.results]
    full = np.stack([outs[2 * b] + outs[2 * b + 1] for b in range(B)])
    if _trace:
        return full.astype(np.float32), res
    return full.astype(np.float32)



# revision 35
# speedup vs baseline: 1.0303x; 1.0303x over previous
"""Causal self-attention (RMS-normed QK, RoPE, GQA) Trainium2 Bass kernel.

Sharding over 8 NeuronCores: 4-way data-parallel over batch x 2-way
tensor-parallel over heads.  Core c handles batch b = c // 2 and head group
g = c % 2 (q heads g*8..g*8+7, kv heads g*2, g*2+1).  Each core produces a
partial output projection; the host sums the two head-group partials per
batch.

v3 design (bf16 data path, fp32 PSUM accumulation everywhere).  The PE
stream is almost pure "useful" matmuls; every side computation rides a
different engine:
  - Host pre-transposes/packs x^T and all weights into the SBUF-native
    [128, ...] partition-major layout, so every DMA is a single contiguous
    [128, N] copy (no on-device transposes of x, no rearrange DMAs).
  - RoPE half-swap as an SBUF->SBUF DMA across partitions, with the sign
    flip folded into the host-built sin table (no PE swap matmul).
  - RMS-norm sum-of-squares via gpsimd partition_all_reduce (result lands
    broadcast on all partitions), Sqrt on the scalar engine, reciprocal
    on vector: no PE ones-matmul, no K=1 broadcast matmul.
  - q_gain/sqrt(hd) ride in the per-head `scale` operand of the Exp.
  - Softmax denominators accumulate on the PE (ones-column matmuls into a
    [1,512] PSUM region); 1/l is broadcast back over partitions by
    gpsimd partition_broadcast.
  - Causal diagonal j-tiles compute only the live columns (s/exp/l/y all
    column-restricted); fully-masked column ranges are never touched.
  - xt streams in e-quarters on the scalar HWDGE ring so the first
    projection starts ~2us in; K and V chunks interleave per column
    block to cover the cold-start DMA; out_w prefetches during
    attention.
  - Attention runs i-block-major; the output projection for each i-block
    is issued right after it, so projection matmuls fill the softmax
    pipeline bubbles and the output DMA streams throughout.  Its PSUM
    tiles share the y-pool slots to stay within 8 banks.
  - y^T stays in SBUF in bf16 and feeds the output projection directly as
    the stationary operand (no DRAM spill).
"""

import math

import numpy as np
import ml_dtypes

import concourse.bass as bass
import concourse.mybir as mybir
import concourse.tile as tile
from concourse import bacc, bass_utils
from concourse.bass_isa import ReduceOp
from concourse.masks import make_identity

F32 = mybir.dt.float32
BF16 = mybir.dt.bfloat16
BF16NP = ml_dtypes.bfloat16

HEAD_DIM = 128
N_HEADS = 16
N_KV_HEADS = 4
ROPE_BASE = 10000.0
TRAIN_SEQ_LEN = 1024

B, D = 4, 2048
H_LOC = 8  # q heads per core
KV_LOC = 2  # kv heads per core
EC = D // 128  # contraction chunks
EPS = float(np.finfo(np.float32).eps)
INV_SQRT_HD = 1.0 / math.sqrt(HEAD_DIM)
AF = mybir.ActivationFunctionType


def _rope_tables(T):
    rd = HEAD_DIM
    base = ROPE_BASE
    if T > TRAIN_SEQ_LEN:
        scale = T / TRAIN_SEQ_LEN
        base = base * scale ** (rd / (rd - 2))
    inv_freq = 1.0 / base ** (np.arange(0, rd, 2, dtype=np.float32) / rd)
    freqs = np.outer(np.arange(T, dtype=np.float32), inv_freq)
    return np.cos(freqs).astype(np.float32), np.sin(freqs).astype(np.float32)


def _blob_layout(T):
    """(name, n_bf16_elements) regions of the packed input blob."""
    return [
        ("xt", 128 * EC * T),
        ("qwt", 128 * H_LOC * EC * 128),
        ("kwt", 128 * KV_LOC * EC * 128),
        ("vwt", 128 * KV_LOC * EC * 128),
        ("owt", 128 * H_LOC * D),
        ("cos2", 128 * T),
        ("sin2", 128 * T),  # sign-folded: lower half +sin, upper half -sin
        ("gains", 2 * H_LOC),  # H_LOC f32 values as raw bf16 pairs
    ]


def build_program(T=2048, phases=(1, 2, 3)):
    """Build the per-core Bass program. T must be a multiple of 512."""
    assert T % 512 == 0
    NT = T // 128  # 128-wide t tiles
    NTB = T // 512  # projection column chunks
    NIB = T // 512  # attention i blocks

    nc = bacc.Bacc("TRN2", target_bir_lowering=False, debug=False, num_devices=8)

    # All inputs live in ONE flat bf16 blob (a single runtime buffer per
    # call is measurably cheaper to dispatch through the runtime than ten).
    sizes = _blob_layout(T)
    total = sum(n for _, n in sizes)
    blob_d = nc.dram_tensor("blob", [total], BF16, kind="ExternalInput").ap()
    regions = {}
    off = 0
    for name, n in sizes:
        regions[name] = blob_d[off:off + n]
        off += n

    def blob_ap(name, free_shape):
        """region as a [128, *free_shape] partition-major AP (C order)."""
        r = regions[name]
        dims = list(free_shape)
        strides = []
        s = 1
        for d in reversed(dims):
            strides.append((s, d))
            s *= d
        strides.reverse()
        ap = [[s, 128]] + [[st, d] for st, d in strides]
        return bass.AP(tensor=r.tensor, offset=r.offset, ap=ap)

    xt_d = blob_ap("xt", (EC, T))
    qwt_d = blob_ap("qwt", (H_LOC, EC, 128))
    kwt_d = blob_ap("kwt", (KV_LOC, EC, 128))
    vwt_d = blob_ap("vwt", (KV_LOC, EC, 128))
    owt_d = blob_ap("owt", (H_LOC, D))
    cos_d = blob_ap("cos2", (T,))
    sin_d = blob_ap("sin2", (T,))
    g_r = regions["gains"]  # [2*H_LOC] bf16 holding H_LOC f32 values
    gains_d = bass.AP(tensor=g_r.tensor, offset=g_r.offset,
                      ap=[[0, 128], [1, 2 * H_LOC]])
    out_d = nc.dram_tensor("out", [T, D], F32, kind="ExternalOutput").ap()

    with tile.TileContext(nc) as tc:
        with (
            tc.tile_pool(name="const", bufs=1) as const_p,
            tc.tile_pool(name="pers", bufs=1) as pers_p,
        ):
            ident = const_p.tile([128, 128], BF16)
            make_identity(nc, ident)
            ones_p = const_p.tile([128, 1], BF16)  # lhsT for partition-sum
            nc.vector.memset(ones_p, 1.0)
            eps_sb = const_p.tile([128, 1], F32)
            nc.vector.memset(eps_sb, EPS)
            cos_sb = const_p.tile([128, T], BF16)
            sin_sb = const_p.tile([128, T], BF16)
            graw_sb = const_p.tile([128, 2 * H_LOC], BF16)
            gscl_sb = const_p.tile([128, H_LOC], F32)
            # causal 0/1 masks for the 4 diagonal j-tiles of an i-block:
            # mask[p, v, c] = 1 iff c >= 128*v + p
            mask_sb = const_p.tile([128, 4, 512], BF16)
            nc.vector.memset(mask_sb, 1.0)
            nc.gpsimd.affine_select(
                out=mask_sb, in_=mask_sb,
                compare_op=mybir.AluOpType.is_ge, fill=0.0,
                base=0, channel_multiplier=-1,
                pattern=[[-128, 4], [1, 512]])

            qT = pers_p.tile([128, H_LOC, T], BF16)
            kT = pers_p.tile([128, KV_LOC, T], BF16)
            v_sb = pers_p.tile([128, NT, KV_LOC * 128], BF16)
            yT = pers_p.tile([128, H_LOC, T], BF16)

            # ---------------- Phase 1: projections -----------------------
            with (
                tc.tile_pool(name="p1xt", bufs=1) as xt_p,
                tc.tile_pool(name="p1w", bufs=2) as w_p,
                tc.tile_pool(name="p1wk", bufs=3) as wk_p,
                tc.tile_pool(name="p1vt", bufs=1) as vt_p,
                tc.tile_pool(name="p1row", bufs=2) as row_p,
                tc.tile_pool(name="p1psh", bufs=4, space="PSUM") as ps_h,
                tc.tile_pool(name="p1psq", bufs=3, space="PSUM") as ps_q,
            ):
                def load_w(w_dram, idx):
                    wt = w_p.tile([128, EC, 128], BF16, tag="w")
                    nc.sync.dma_start(wt, w_dram[:, idx, :, :])
                    return wt

                # first weight before the big constants, so the first
                # projection matmul is not queued behind cos/sin
                wt_first = load_w(kwt_d, 0)

                xt = xt_p.tile([128, EC, T], BF16)
                # all xt loads on the scalar HWDGE ring in e-quarters so
                # projections start as soon as their contraction slices
                # land; Pool stays free for partition_all_reduce
                for tb in range(NTB):
                    tsl = slice(tb * 512, (tb + 1) * 512)
                    for part in range(4):
                        esl = slice(4 * part, 4 * part + 4)
                        nc.scalar.dma_start(xt[:, esl, tsl], xt_d[:, esl, tsl])

                nc.sync.dma_start(cos_sb, cos_d)
                nc.sync.dma_start(sin_sb, sin_d)
                # gains arrive as raw f32 bits inside the bf16 blob;
                # broadcast to all partitions via stride-0 DMA, then scale.
                nc.sync.dma_start(graw_sb, gains_d)
                nc.scalar.mul(gscl_sb, graw_sb.bitcast(F32), INV_SQRT_HD)

                def project_chunk(wt, tsl):
                    h_ps = ps_h.tile([128, 512], F32, tag="hps")
                    for e in range(EC):
                        nc.tensor.matmul(h_ps, wt[:, e, :], xt[:, e, tsl],
                                         start=(e == 0), stop=(e == EC - 1))
                    return h_ps

                def norm_rope_chunk(h_ps, tsl, dst):
                    """dst = rms_norm+rope of the raw projection chunk.

                    No PE instructions: the partition-sum runs on GPSIMD,
                    the rope half-swap is an SBUF-SBUF DMA across
                    partitions (sign folded into the sin table), so the PE
                    stream is pure projection matmuls.
                    """
                    x_sb = wk_p.tile([128, 512], BF16, tag="xsb")
                    nc.scalar.copy(x_sb, h_ps)
                    sq = wk_p.tile([128, 512], BF16, tag="sq")
                    nc.scalar.square(sq, h_ps)
                    ssq = row_p.tile([128, 512], F32, tag="ssq")
                    nc.gpsimd.partition_all_reduce(ssq, sq, 128, ReduceOp.add)
                    rms = wk_p.tile([128, 512], BF16, tag="rms")
                    with nc.allow_low_precision(reason="bf16 norm scale"):
                        nc.scalar.activation(rms, ssq, AF.Sqrt,
                                             bias=eps_sb, scale=1.0 / 128.0)
                    rinv = wk_p.tile([128, 512], BF16, tag="rinv")
                    with nc.allow_low_precision(reason="bf16 norm scale"):
                        nc.vector.reciprocal(rinv, rms)
                    # rope: rot = x*cos + swap64(x)*sin', sin' sign-folded
                    xsw = wk_p.tile([128, 512], BF16, tag="xsw")
                    nc.sync.dma_start(xsw[0:64, :], x_sb[64:128, :])
                    nc.sync.dma_start(xsw[64:128, :], x_sb[0:64, :])
                    rc = wk_p.tile([128, 512], BF16, tag="rc")
                    nc.vector.tensor_mul(rc, x_sb, cos_sb[:, tsl])
                    qsw = wk_p.tile([128, 512], BF16, tag="qsw")
                    nc.vector.tensor_mul(qsw, xsw, sin_sb[:, tsl])
                    qr = wk_p.tile([128, 512], BF16, tag="qr")
                    nc.vector.tensor_add(qr, rc, qsw)
                    nc.vector.tensor_mul(dst, qr, rinv)

                for kv in range(KV_LOC):
                    # K and V chunks interleave per column block so the PE
                    # has ~7us of work per 6.3us xt column-block DMA during
                    # the cold start.  V transposes trail after each kv
                    # head, overlapping the next head's projections.
                    wt = wt_first if kv == 0 else load_w(kwt_d, kv)
                    wtv = load_w(vwt_d, kv)
                    vts = []
                    for tb in range(NTB):
                        tsl = slice(tb * 512, (tb + 1) * 512)
                        h_ps = project_chunk(wt, tsl)
                        norm_rope_chunk(h_ps, tsl, kT[:, kv, tsl])
                        v_ps = project_chunk(wtv, tsl)
                        vt = vt_p.tile([128, 512], BF16, tag=f"vt{tb}")
                        nc.vector.tensor_copy(vt, v_ps)
                        vts.append(vt)
                    for tb in range(NTB):
                        for tt in range(4):
                            pst = ps_q.tile([128, 128], BF16, tag="qsps")
                            nc.tensor.transpose(
                                pst, vts[tb][:, tt * 128:(tt + 1) * 128],
                                ident)
                            nc.vector.tensor_copy(
                                v_sb[:, tb * 4 + tt,
                                     kv * 128:(kv + 1) * 128], pst)

                for h in range(H_LOC):
                    wt = load_w(qwt_d, h)
                    for tb in range(NTB):
                        tsl = slice(tb * 512, (tb + 1) * 512)
                        h_ps = project_chunk(wt, tsl)
                        norm_rope_chunk(h_ps, tsl, qT[:, h, tsl])

            # out_w prefetch: pool opens after phase-1 pools free their
            # SBUF, DMA overlaps the whole attention phase.
            with tc.tile_pool(name="p3ow", bufs=1) as ow_p:
                ow_sb = ow_p.tile([128, H_LOC, D], BF16)
                nc.sync.dma_start(ow_sb, owt_d)

                # ------- Phase 2+3: attention + output projection ---------
                # ib-major attention; after each i-block completes for all
                # heads, the output projection for those 4 i-tiles is
                # issued.  The projection matmuls sit in the PE queue with
                # all inputs ready, filling the softmax-chain stalls at
                # block boundaries, and the output DMA streams throughout
                # instead of all at the end.  o_ps shares the y-pool slots
                # (same shape/tag) to stay within 8 PSUM banks.
                with (
                    tc.tile_pool(name="p2pt", bufs=4) as pt_p,
                    tc.tile_pool(name="p2y", bufs=2) as ystg_p,
                    tc.tile_pool(name="p2row", bufs=2) as row2_p,
                    tc.tile_pool(name="p3o", bufs=2) as ostg_p,
                    tc.tile_pool(name="p2pss", bufs=4, space="PSUM") as ps_s,
                    tc.tile_pool(name="p2psy", bufs=2, space="PSUM") as ps_y,
                    tc.tile_pool(name="p2psl", bufs=2, space="PSUM") as ps_l,
                ):
                    for ib in range(NIB if 2 in phases else 0):
                        jmax = 4 * ib + 3
                        isl = slice(ib * 512, (ib + 1) * 512)
                        for h in range(H_LOC):
                            kv = h // (N_HEADS // N_KV_HEADS)
                            y_ps = ps_y.tile([128, 512], F32, tag="y")
                            l_ps = ps_l.tile([1, 512], F32, tag="l")
                            for jt in range(jmax + 1):
                                # diagonal j-tiles: columns below the causal
                                # front are all-masked; skip them entirely
                                v = jt - 4 * ib
                                c0 = 128 * v if v > 0 else 0
                                cols = slice(c0, 512)
                                qsl = slice(ib * 512 + c0, (ib + 1) * 512)
                                s_ps = ps_s.tile([128, 512], F32, tag="s")
                                nc.tensor.matmul(
                                    s_ps[:, cols],
                                    kT[:, kv, jt * 128:(jt + 1) * 128],
                                    qT[:, h, qsl], start=True, stop=True)
                                pt = pt_p.tile([128, 512], BF16, tag="pt")
                                nc.scalar.activation(
                                    pt[:, cols], s_ps[:, cols], AF.Exp,
                                    scale=gscl_sb[:, h:h + 1])
                                ptu = pt
                                if v >= 0:  # diagonal j-tile: causal mask
                                    ptm = pt_p.tile([128, 512], BF16,
                                                    tag="ptm")
                                    nc.vector.tensor_mul(
                                        ptm[:, cols], pt[:, cols],
                                        mask_sb[:, v, cols])
                                    ptu = ptm
                                nc.tensor.matmul(
                                    l_ps[:, cols], ones_p, ptu[:, cols],
                                    start=(jt == 0), stop=(jt == jmax))
                                nc.tensor.matmul(
                                    y_ps[:, cols],
                                    v_sb[:, jt, kv * 128:(kv + 1) * 128],
                                    ptu[:, cols],
                                    start=(jt == 0), stop=(jt == jmax))
                            lrow = row2_p.tile([1, 512], BF16, tag="lr")
                            with nc.allow_low_precision(reason="bf16 softmax"):
                                nc.vector.reciprocal(lrow, l_ps)
                            linv_bc = ystg_p.tile([128, 512], BF16, tag="linv")
                            nc.gpsimd.partition_broadcast(linv_bc, lrow)
                            nc.vector.tensor_mul(yT[:, h, isl], y_ps, linv_bc)

                        if 3 not in phases:
                            continue
                        for it in range(4 * ib, 4 * ib + 4):
                            o_sb = ostg_p.tile([128, D], F32, tag="osb")
                            for db in range(D // 512):
                                o_ps = ps_y.tile([128, 512], F32, tag="y")
                                for hh in range(H_LOC):
                                    nc.tensor.matmul(
                                        o_ps,
                                        yT[:, hh, it * 128:(it + 1) * 128],
                                        ow_sb[:, hh,
                                              db * 512:(db + 1) * 512],
                                        start=(hh == 0),
                                        stop=(hh == H_LOC - 1))
                                nc.vector.tensor_copy(
                                    o_sb[:, db * 512:(db + 1) * 512], o_ps)
                            nc.sync.dma_start(
                                out_d[it * 128:(it + 1) * 128, :], o_sb)

    nc.compile()
    return nc


def _pack_weight(w):
    """w [ncols, D]: returns [128, nh, EC, 128] with
    out[p, h, e, c] = w[h*128 + c, e*128 + p]."""
    nh = w.shape[0] // 128
    return np.ascontiguousarray(
        w.reshape(nh, 128, EC, 128).transpose(3, 0, 2, 1)).astype(BF16NP)


def make_in_maps(x, q_w, k_w, v_w, out_w, q_gain, T):
    cos, sin = _rope_tables(T)
    cosT = np.ascontiguousarray(cos.T)  # [64, T]
    sinT = np.ascontiguousarray(sin.T)
    cos2 = np.concatenate([cosT, cosT], axis=0).astype(BF16NP)  # [128, T]
    # rope sign folded into the table: rot = x*cos2 + swap64(x)*sin2
    sin2 = np.concatenate([sinT, -sinT], axis=0).astype(BF16NP)

    layout = _blob_layout(T)
    in_maps = []
    for c in range(8):
        b, g = c // 2, c % 2
        # xt[p, e, t] = x[b][t, e*128+p]
        xtp = np.ascontiguousarray(
            x[b].T.reshape(EC, 128, T).transpose(1, 0, 2)).astype(BF16NP)
        # ow[p, h, d] = out_w[d, g*1024 + h*128 + p]
        owp = np.ascontiguousarray(
            out_w[:, g * 1024:(g + 1) * 1024].T
            .reshape(H_LOC, 128, D).transpose(1, 0, 2)).astype(BF16NP)
        parts = {
            "xt": xtp,
            "qwt": _pack_weight(q_w[g * 1024:(g + 1) * 1024, :]),
            "kwt": _pack_weight(k_w[g * 256:(g + 1) * 256, :]),
            "vwt": _pack_weight(v_w[g * 256:(g + 1) * 256, :]),
            "owt": owp,
            "cos2": cos2,
            "sin2": sin2,
            "gains": np.ascontiguousarray(
                q_gain[g * H_LOC:(g + 1) * H_LOC]).astype(np.float32)
                .view(BF16NP),
        }
        blob = np.concatenate(
            [np.asarray(parts[name]).reshape(-1) for name, _ in layout])
        for (name, n), arr in zip(layout, [parts[n] for n, _ in layout]):
            assert np.asarray(arr).size == n, (name, np.asarray(arr).size, n)
        in_maps.append({"blob": blob})
    return in_maps


def kernel(x, q_w, k_w, v_w, out_w, q_gain, _trace=False, _trace_cores=None):
    x = np.asarray(x, dtype=np.float32)
    q_w = np.asarray(q_w, dtype=np.float32)
    k_w = np.asarray(k_w, dtype=np.float32)
    v_w = np.asarray(v_w, dtype=np.float32)
    out_w = np.asarray(out_w, dtype=np.float32)
    q_gain = np.asarray(q_gain, dtype=np.float32)
    T = x.shape[1]

    nc = build_program(T)
    in_maps = make_in_maps(x, q_w, k_w, v_w, out_w, q_gain, T)
    res = bass_utils.run_bass_kernel_spmd(
        nc, in_maps, core_ids=list(range(8)),
        trace=_trace, trace_cores=_trace_cores)
    outs = [r["out"] for r in res.results]
    full = np.stack([outs[2 * b] + outs[2 * b + 1] for b in range(B)])
    if _trace:
        return full.astype(np.float32), res
    return full.astype(np.float32)

